# revision 91
# baseline (speedup 1.0000x reference)
"""Power attention (p=2) layer for Trainium2, 8 NeuronCores.

Math: spow2(q).spow2(k) = (q.k)^2, so the reference recurrence equals
    out_t = sum_{s<=t} g^(t-s) (q_t.k_s)^2 v_s   (masked quadratic attention).

Decay: LOCAL per-128-tile scaling. With ti = t mod 128, si = s mod 128:
  qT cols scaled by g^(ti/2), kT cols by g^(-si/2)  ->  diag tile products
  carry exactly g^(ti-si) after squaring. The band tile (k-tile a vs query
  tile a+1, first 64 query cols) uses kT2 = kT * g^64, giving g^(128+tb-si)
  after squaring. Window 192 per k-tile: dropped terms decay <= g^65 ~ 1e-3.

Upper-tri invalid pairs of the diag tile are squared (may overflow f16 to
inf harmlessly) then REPLACED with 0 by a gpsimd affine_select.

Sharding: core c -> batch c//4, head group c%4 (4 heads = 128 qkv cols).
Host sums the 4 partial [S, HIDDEN] outputs per batch and adds o_b.

All DRAM traffic is f16; PSUM accumulation f32.
"""

import math
import sys

import numpy as np

sys.path.insert(0, "/opt/trn_rl_repo")

import concourse.bass as bass  # noqa: E402
import concourse.tile as tile  # noqa: E402
from concourse import bacc  # noqa: E402
from concourse import mybir  # noqa: E402
from concourse import bass_utils  # noqa: E402
from concourse.bass import ts  # noqa: E402

B, S, HIDDEN = 2, 1024, 512
NH, HD = 16, 32
GAMMA = 0.9
NCORES = 8
HPC = 4            # heads per core
CW = HPC * HD      # 128 qkv columns per core
NKT = HIDDEN // 128  # 4 contraction tiles
NST = S // 128       # 8 seq tiles
STRIP = 512
NSTRIP = S // STRIP  # 2
BAND = 64            # band width beyond the diag tile (window = 128+BAND)
WIN = 128 + BAND

G64 = float(GAMMA) ** 64

F32 = mybir.dt.float32
F16 = mybir.dt.float16
AF = mybir.ActivationFunctionType
OP = mybir.AluOpType

# ---- tuning knobs ----
N_WARMUP = 1          # warmup matmuls of 512 rows each (pstate ramp cover)
WARM_ROWS = 128


def _free_bcast(ap2d, times):
    """[P, N] AP -> [P, times, N] AP broadcasting along a new middle free dim."""
    part, free = ap2d.ap[0], list(ap2d.ap[1:])
    return bass.AP(tensor=ap2d.tensor, offset=ap2d.offset,
                   ap=[part, [0, times]] + free)


def _bcast_dram(ap, p=128):
    """DRAM-side AP replicating a [1, ...] tensor across p partitions."""
    return bass.AP(tensor=ap.tensor, offset=ap.offset,
                   ap=[[0, p]] + list(ap.ap[1:]))


def _build_program():
    nc = bacc.Bacc("TRN2", debug=False, target_bir_lowering=False)

    # DRAM inputs (per core, host-prepared, all f16)
    xT = nc.dram_tensor("xT", [128, NKT, S], F16, kind="ExternalInput").ap()
    wk = nc.dram_tensor("wk", [128, NKT, 3 * CW], F16, kind="ExternalInput").ap()
    # misc: [qkvb(6: f32 qb,kb,vb bitcast) | gkh(2: f32 g^(-si/2) bitcast)
    #        | ident(128) | dtri(128) | ow(512)]
    misc = nc.dram_tensor("misc", [CW, 8 + 384 + HIDDEN], F16,
                          kind="ExternalInput").ap()
    # head: first-use bundle {wk ktile0 | wk ktile1 | xT ktile0 strip0}
    head = nc.dram_tensor("head", [128, 2 * 3 * CW + STRIP], F16,
                          kind="ExternalInput").ap()
    yp = nc.dram_tensor("yp", [S, HIDDEN], F16, kind="ExternalOutput").ap()

    with tile.TileContext(nc) as tc:
        with (
            tc.tile_pool(name="const", bufs=1) as const,
            tc.tile_pool(name="apool", bufs=4) as apool,
            tc.tile_pool(name="ypool", bufs=8) as ypool,
            tc.tile_pool(name="mmp", bufs=3, space="PSUM") as mmp,
            tc.tile_pool(name="scp", bufs=2, space="PSUM") as scp,
            tc.tile_pool(name="otp", bufs=1, space="PSUM") as otp,
        ):
            # ---------------- DMAs (order = HWDGE priority) ----------------
            wk_sb = const.tile([128, NKT, 3 * CW], F16)
            xT_sb = const.tile([128, NKT, S], F16)
            misc_sb = const.tile([CW, 8 + 384 + HIDDEN], F16)

            head_sb = const.tile([128, 2 * 3 * CW + STRIP], F16)

            xr = xT  # dram layout already [128, NKT, S]
            # head bundle first (wk k0/k1 + xT k0-s0: the first 1.3us of PE
            # work in one DMA); miscA (biases+gkh) early; miscB later
            nc.sync.dma_start(head_sb, head)
            nc.scalar.dma_start(xT_sb[:, 1, ts(0, STRIP)], xr[:, 1, ts(0, STRIP)])
            nc.sync.dma_start(wk_sb[:, 2:4, :], wk[:, 2:4, :])
            nc.scalar.dma_start(misc_sb[:, 0:136], misc[:, 0:136])
            nc.sync.dma_start(xT_sb[:, 2, ts(0, STRIP)], xr[:, 2, ts(0, STRIP)])
            nc.scalar.dma_start(xT_sb[:, 3, ts(0, STRIP)], xr[:, 3, ts(0, STRIP)])
            nc.scalar.dma_start(misc_sb[:, 136:], misc[:, 136:])
            nc.sync.dma_start(xT_sb[:, :, ts(1, STRIP)], xr[:, :, ts(1, STRIP)])

            wkap = [head_sb[:, 0:384], head_sb[:, 384:768],
                    wk_sb[:, 2, :], wk_sb[:, 3, :]]
            x0ap = [head_sb[:, 768: 768 + STRIP], xT_sb[:, 1, ts(0, STRIP)],
                    xT_sb[:, 2, ts(0, STRIP)], xT_sb[:, 3, ts(0, STRIP)]]

            qkvb_sb = misc_sb[:, 0:6].bitcast(F32)      # [128, 3] f32
            qb_sb = qkvb_sb[:, 0:1]
            kb_sb = qkvb_sb[:, 1:2]
            vb_sb = qkvb_sb[:, 2:3]
            gkh_sb = misc_sb[:, 6:8].bitcast(F32)       # [128, 1] f32 g^(-p/2)
            gq_sb = misc_sb[:, 8:136]
            ident_sb = misc_sb[:, 136:264]
            dtri_sb = misc_sb[:, 264:392]
            ow_sb = misc_sb[:, 392: 392 + HIDDEN]



            qT_sb = const.tile([CW, S], F16, tag="qT")
            kT_sb = const.tile([CW, S], F16, tag="kT")
            kT2_sb = const.tile([CW, S], F16, tag="kT2")
            vT_sb = const.tile([CW, S], F16, tag="vT")
            v_sb = const.tile([128, NST, CW], F16, tag="v")
            outT_sb = const.tile([CW, S], F16, tag="outT")
            warm_sb = const.tile([128, WARM_ROWS], F16, tag="warm")

            # ------------- warmup: PE pstate ramp + Act table load ----------
            nc.gpsimd.memset(warm_sb, 0.0)
            warm_ps = mmp.tile([128, 512], F32, tag="mm", name="warm")
            for i in range(N_WARMUP):
                nc.tensor.matmul(warm_ps[:, 0:WARM_ROWS], warm_sb[:, 0:128],
                                 warm_sb[:, 0:WARM_ROWS], start=True, stop=True)
            # Act function-table load early (Square/Identity/Copy set)
            nc.scalar.square(warm_sb[:, 0:64], warm_sb[:, 64:128])



            # ---------------- QKV projection for one 512-strip --------------
            # per-ktile interleave: q,k,v consume each xT ktile as it lands
            def qkv_strip(T):
                tsl = ts(T, STRIP)
                g2 = lambda g: _free_bcast(g, STRIP // 128)

                # sequential proj groups: k closes first (scores path), then
                # q, then v — staggers the psum->SBUF copies across engines
                kps = mmp.tile([128, STRIP], F32, tag="mm", name=f"k{T}")
                xv = lambda kk: x0ap[kk] if T == 0 else xT_sb[:, kk, tsl]
                for kk in range(NKT):
                    nc.tensor.matmul(kps, wkap[kk][:, CW:2 * CW], xv(kk),
                                     start=kk == 0, stop=kk == NKT - 1)
                # k decay g^(-si/2) is applied later as the Square's scale
                nc.scalar.activation(kT_sb[:, tsl], kps, AF.Identity, bias=kb_sb)
                qps = mmp.tile([128, STRIP], F32, tag="mm", name=f"q{T}")
                for kk in range(NKT):
                    nc.tensor.matmul(qps, wkap[kk][:, 0:CW], xv(kk),
                                     start=kk == 0, stop=kk == NKT - 1)
                nc.vector.scalar_tensor_tensor(
                    out=qT_sb[:, tsl].rearrange("p (a b) -> p a b", b=128),
                    in0=qps.rearrange("p (a b) -> p a b", b=128),
                    scalar=qb_sb, in1=g2(gq_sb), op0=OP.add, op1=OP.mult)
                # band stationary: kT2 = kT * g^64 (DVE 4x mode, all f16 SBUF)
                nc.vector.tensor_scalar_mul(kT2_sb[:, tsl], kT_sb[:, tsl], G64)
                vps = mmp.tile([128, STRIP], F32, tag="mm", name=f"v{T}")
                for kk in range(NKT):
                    nc.tensor.matmul(vps, wkap[kk][:, 2 * CW:3 * CW], xv(kk),
                                     start=kk == 0, stop=kk == NKT - 1)
                nc.scalar.activation(vT_sb[:, tsl], vps, AF.Identity,
                                     bias=vb_sb)

            def transpose_v(a):
                tp = mmp.tile([128, 128], F16, tag="mm", name=f"tp{a}")
                nc.tensor.transpose(tp, vT_sb[:, ts(a, 128)], ident_sb)
                nc.vector.tensor_copy(v_sb[:, a, :], tp)

            # ---------------- attention tile a ------------------------------
            oT = otp.tile([128, 2, 256], F32, tag="oT")  # one bank, 2 strips

            a4s = {}

            def att_sc(a, mask_dve=False):
                has_band = a < NST - 1
                n = WIN if has_band else 128
                w0 = 128 * a
                # scores psum: [128, 2, 512]-stride tile = 2 banks, head h
                # region [:, hh, :] gets its own bank (same-bank row-group
                # pairs hang)
                a4 = apool.tile([128, HPC, WIN], F16, tag="a4", name=f"a4_{a}")
                a4s[a] = a4
                for g, ps2 in ((0, scp.tile([128, 2, 512], F32, tag="sc",
                                            name=f"scA{a}")),
                               (1, scp.tile([128, 2, 512], F32, tag="sc",
                                            name=f"scB{a}"))):
                    for hh in range(2):
                        h = 2 * g + hh
                        hs = ts(h, 32)
                        # diag: stationary kT tile-a, moving qT same window
                        nc.tensor.matmul(
                            ps2[:, hh, 0:128], kT_sb[hs, ts(a, 128)],
                            qT_sb[hs, ts(a, 128)],
                            start=True, stop=True,
                            tile_position=(32 * h, 0))
                        if has_band:
                            # band: stationary kT2 (g^64 folded), moving qT of
                            # next tile's first 64 cols
                            nc.tensor.matmul(
                                ps2[:, hh, 128:WIN], kT2_sb[hs, ts(a, 128)],
                                qT_sb[hs, w0 + 128: w0 + WIN],
                                start=True, stop=True,
                                tile_position=(32 * h, 0),
                                skip_group_check=True)
                    nc.scalar.activation(a4[:, 2 * g: 2 * g + 2, 0:n],
                                         ps2[:, :, 0:n], AF.Square,
                                         scale=gkh_sb)
                    # zero the invalid (si > ti) upper part of the diag
                    # block; per head-group so av h01 starts after group 0
                    nc.gpsimd.affine_select(
                        a4[:, 2 * g: 2 * g + 2, 0:128],
                        a4[:, 2 * g: 2 * g + 2, 0:128],
                        pattern=[[0, 2], [1, 128]], compare_op=OP.is_ge,
                        fill=0.0, base=0, channel_multiplier=-1)

            # block j7 (t 896:1024) accumulates in a separate bank (borrowed
            # from the sc pool) so j6 and j7 close independently at the tail
            ot3b = {}

            def att_av(a):
                has_band = a < NST - 1
                a4 = a4s.pop(a)
                # A@V accumulation into oT strips (memset + pure accumulate)
                Td, cd = divmod(a, 2)     # diag -> strip Td cols [128*cd, +128)
                Tb, cb = divmod(a + 1, 2)  # band -> strip Tb cols [128*cb, +64)
                # band first: its a4 region needs no mask, so these matmuls
                # issue while the diag mask is still in flight
                if has_band:
                    bb = oT[:, Tb % 2, 128 * cb: 128 * cb + BAND]
                    if a == 6:
                        bb = ot3b["t"][:, 0:BAND]
                    for h in range(HPC):
                        hs = ts(h, 32)
                        nc.tensor.matmul(
                            bb[hs, :],
                            v_sb[:, a, hs], a4[:, h, 128:WIN], start=False,
                            stop=False,
                            tile_position=(0, 32 * h), skip_group_check=True)
                dd = oT[:, Td % 2, ts(cd, 128)]
                if a == 7:
                    dd = ot3b["t"][:, 0:128]
                for h in range(HPC):
                    hs = ts(h, 32)
                    nc.tensor.matmul(
                        dd[hs, :], v_sb[:, a, hs],
                        a4[:, h, 0:128], start=False,
                        stop=(h == HPC - 1 and a in (2 * Td + 1, 6, 7)),
                        tile_position=(0, 32 * h), skip_group_check=True)

            def memset_strip(T):
                nc.vector.memset(oT[:, T % 2, :], 0.0)

            def close_strip(T, split=False):
                base = 256 * T
                nc.vector.tensor_copy(outT_sb[:, base: base + 256],
                                      oT[:, T % 2, :])

            def outproj(j, qidx, copy="dve"):
                yps = mmp.tile([128, HIDDEN], F32, tag="mm", name=f"y{j}")
                nc.tensor.matmul(yps, outT_sb[:, ts(j, 128)], ow_sb,
                                 start=True, stop=True)
                y_sb = ypool.tile([128, HIDDEN], F16, tag="y")
                if copy == "split":
                    nc.vector.tensor_copy(y_sb[:, 0:256], yps[:, 0:256])
                    nc.scalar.copy(y_sb[:, 256:512], yps[:, 256:512])
                elif copy == "act":
                    nc.scalar.copy(y_sb, yps)
                else:
                    nc.vector.tensor_copy(y_sb, yps)
                q = nc.scalar if j == 6 else nc.sync
                q.dma_start(yp[ts(j, 128), :], y_sb)

            # ---------------- schedule ----------------
            # att split in two phases (sc -> av) with 2 tiles of lookahead so
            # the in-order PE queue hides the square->mask chain latency.
            memset_strip(0)
            memset_strip(1)
            qkv_strip(0)
            att_sc(0)
            att_sc(1)
            transpose_v(0)
            transpose_v(1)
            att_sc(2)
            transpose_v(2)
            transpose_v(3)
            att_av(0)
            qkv_strip(1)
            att_sc(3)
            att_av(1)
            close_strip(0)
            memset_strip(2)
            att_sc(4)
            outproj(0, 0)
            att_av(2)
            outproj(1, 1)
            transpose_v(4)
            transpose_v(5)
            transpose_v(6)
            transpose_v(7)
            att_sc(5)
            att_av(3)
            close_strip(1)
            memset_strip(3)
            att_sc(6)
            outproj(2, 0)
            att_av(4)
            outproj(3, 1)
            att_sc(7)
            att_av(5)
            close_strip(2)
            outproj(4, 0)
            outproj(5, 1)
            ot3b["t"] = mmp.tile([128, 512], F32, tag="mm", name="ot3b")
            nc.vector.memset(ot3b["t"][:, 0:128], 0.0)
            att_av(6)
            # close j6 (oT strip3 low half) independently of av(7)
            nc.scalar.copy(outT_sb[:, 768:896], oT[:, 1, 0:128])
            outproj(6, 1, copy="act")
            att_av(7)
            nc.vector.tensor_copy(outT_sb[:, 896:1024], ot3b["t"][:, 0:128])
            outproj(7, 0, copy="dve")

    nc.compile()
    return nc


_CACHED = None


def _get_program():
    global _CACHED
    if _CACHED is None:
        _CACHED = _build_program()
    return _CACHED


def _in_maps(x, q_w, q_b, k_w, k_b, v_w, v_b, o_w, o_b):
    x = np.asarray(x, np.float32)
    ti = np.arange(128, dtype=np.float64)
    gq = (GAMMA ** (ti / 2)).astype(np.float16)
    gq_rep = np.broadcast_to(gq, (CW, 128))

    qw_f, kw_f, vw_f = (np.asarray(w, np.float32) for w in (q_w, k_w, v_w))
    qb_f, kb_f, vb_f = (np.asarray(b, np.float32) for b in (q_b, k_b, v_b))
    ow_f = np.asarray(o_w, np.float32)
    ident = np.eye(128, dtype=np.float16)

    in_maps = []
    for c in range(NCORES):
        b, g = divmod(c, HPC)
        cs = slice(g * CW, (g + 1) * CW)
        # xT: [hidden, S] -> [NKT, 128, S] -> [128, NKT, S]
        xT_v = np.ascontiguousarray(
            x[b].T.reshape(NKT, 128, S).transpose(1, 0, 2).astype(np.float16))
        wk_v = np.concatenate([qw_f[:, cs], kw_f[:, cs], vw_f[:, cs]], axis=1)
        wk_v = np.ascontiguousarray(
            wk_v.reshape(NKT, 128, 3 * CW).astype(np.float16))
        wk_v = np.ascontiguousarray(wk_v.transpose(1, 0, 2))
        qkvb = np.stack([qb_f[cs], kb_f[cs], vb_f[cs]], axis=1).astype(np.float32)
        qkvb16 = qkvb.view(np.float16).reshape(CW, 6)
        gkh = (GAMMA ** (-np.arange(128, dtype=np.float64) / 2)).astype(
            np.float32)[:, None]
        gkh16 = gkh.view(np.float16).reshape(CW, 2)
        dtri = np.triu(np.ones((128, 128), np.float16))
        misc_v = np.ascontiguousarray(np.concatenate(
            [qkvb16, gkh16, gq_rep, ident, dtri,
             ow_f[cs, :].astype(np.float16)], axis=1))
        head_v = np.ascontiguousarray(np.concatenate(
            [wk_v[:, 0, :], wk_v[:, 1, :], xT_v[:, 0, 0:STRIP]], axis=1))
        in_maps.append({
            "xT": xT_v,
            "wk": wk_v,
            "misc": misc_v,
            "head": head_v,
        })
    return in_maps


def _gather(res, o_b):
    parts = [res.results[c]["yp"] for c in range(NCORES)]
    out = np.empty((B, S, HIDDEN), np.float32)
    ob = np.asarray(o_b, np.float32)
    for b in range(B):
        out[b] = (
            parts[4 * b].astype(np.float32)
            + parts[4 * b + 1].astype(np.float32)
            + parts[4 * b + 2].astype(np.float32)
            + parts[4 * b + 3].astype(np.float32)
            + ob
        )
    return out


def kernel(x, q_w, q_b, k_w, k_b, v_w, v_b, o_w, o_b):
    in_maps = _in_maps(x, q_w, q_b, k_w, k_b, v_w, v_b, o_w, o_b)
    nc = _get_program()
    res = bass_utils.run_bass_kernel_spmd(nc, in_maps, core_ids=list(range(NCORES)))
    return _gather(res, o_b)


def cost_model_time_ns():
    """Per-core makespan from the instruction cost model (no NTFF on axon)."""
    from concourse.timeline_sim import TimelineSim

    return TimelineSim(_get_program(), trace=False).simulate()


if __name__ == "__main__":
    rng = np.random.default_rng(0)
    lim = 1.0 / math.sqrt(HIDDEN)
    ins = {
        "x": rng.standard_normal((B, S, HIDDEN), dtype=np.float32),
        "q_w": rng.uniform(-lim, lim, (HIDDEN, HIDDEN)).astype(np.float32),
        "q_b": rng.uniform(-lim, lim, HIDDEN).astype(np.float32),
        "k_w": rng.uniform(-lim, lim, (HIDDEN, HIDDEN)).astype(np.float32),
        "k_b": rng.uniform(-lim, lim, HIDDEN).astype(np.float32),
        "v_w": rng.uniform(-lim, lim, (HIDDEN, HIDDEN)).astype(np.float32),
        "v_b": rng.uniform(-lim, lim, HIDDEN).astype(np.float32),
        "o_w": rng.uniform(-lim, lim, (HIDDEN, HIDDEN)).astype(np.float32),
        "o_b": rng.uniform(-lim, lim, HIDDEN).astype(np.float32),
    }
    out = kernel(**ins)
    print("kernel ran, out shape", out.shape, "norm", np.linalg.norm(out))


# revision 92
# speedup vs baseline: 1.0015x; 1.0015x over previous
"""Power attention (p=2) layer for Trainium2, 8 NeuronCores.

Math: spow2(q).spow2(k) = (q.k)^2, so the reference recurrence equals
    out_t = sum_{s<=t} g^(t-s) (q_t.k_s)^2 v_s   (masked quadratic attention).

Decay: LOCAL per-128-tile scaling. With ti = t mod 128, si = s mod 128:
  qT cols scaled by g^(ti/2), kT cols by g^(-si/2)  ->  diag tile products
  carry exactly g^(ti-si) after squaring. The band tile (k-tile a vs query
  tile a+1, first 64 query cols) uses kT2 = kT * g^64, giving g^(128+tb-si)
  after squaring. Window 192 per k-tile: dropped terms decay <= g^65 ~ 1e-3.

Upper-tri invalid pairs of the diag tile are squared (may overflow f16 to
inf harmlessly) then REPLACED with 0 by a gpsimd affine_select.

Sharding: core c -> batch c//4, head group c%4 (4 heads = 128 qkv cols).
Host sums the 4 partial [S, HIDDEN] outputs per batch and adds o_b.

All DRAM traffic is f16; PSUM accumulation f32.
"""

import math
import sys

import numpy as np

sys.path.insert(0, "/opt/trn_rl_repo")

import concourse.bass as bass  # noqa: E402
import concourse.tile as tile  # noqa: E402
from concourse import bacc  # noqa: E402
from concourse import mybir  # noqa: E402
from concourse import bass_utils  # noqa: E402
from concourse.bass import ts  # noqa: E402

B, S, HIDDEN = 2, 1024, 512
NH, HD = 16, 32
GAMMA = 0.9
NCORES = 8
HPC = 4            # heads per core
CW = HPC * HD      # 128 qkv columns per core
NKT = HIDDEN // 128  # 4 contraction tiles
NST = S // 128       # 8 seq tiles
STRIP = 512
NSTRIP = S // STRIP  # 2
BAND = 64            # band width beyond the diag tile (window = 128+BAND)
WIN = 128 + BAND

G64 = float(GAMMA) ** 64

F32 = mybir.dt.float32
F16 = mybir.dt.float16
AF = mybir.ActivationFunctionType
OP = mybir.AluOpType

# ---- tuning knobs ----
N_WARMUP = 1          # warmup matmuls of 512 rows each (pstate ramp cover)
WARM_ROWS = 128


def _free_bcast(ap2d, times):
    """[P, N] AP -> [P, times, N] AP broadcasting along a new middle free dim."""
    part, free = ap2d.ap[0], list(ap2d.ap[1:])
    return bass.AP(tensor=ap2d.tensor, offset=ap2d.offset,
                   ap=[part, [0, times]] + free)


def _bcast_dram(ap, p=128):
    """DRAM-side AP replicating a [1, ...] tensor across p partitions."""
    return bass.AP(tensor=ap.tensor, offset=ap.offset,
                   ap=[[0, p]] + list(ap.ap[1:]))


def _build_program():
    nc = bacc.Bacc("TRN2", debug=False, target_bir_lowering=False)

    # DRAM inputs (per core, host-prepared, all f16)
    xT = nc.dram_tensor("xT", [128, NKT, S], F16, kind="ExternalInput").ap()
    wk = nc.dram_tensor("wk", [128, NKT, 3 * CW], F16, kind="ExternalInput").ap()
    # misc: [qkvb(6: f32 qb,kb,vb bitcast) | gkh(2: f32 g^(-si/2) bitcast)
    #        | ident(128) | dtri(128) | ow(512)]
    misc = nc.dram_tensor("misc", [CW, 8 + 384 + HIDDEN], F16,
                          kind="ExternalInput").ap()
    # head: first-use bundle {wk ktile0 | wk ktile1 | xT ktile0 strip0}
    head = nc.dram_tensor("head", [128, 2 * 3 * CW + STRIP], F16,
                          kind="ExternalInput").ap()
    yp = nc.dram_tensor("yp", [S, HIDDEN], F16, kind="ExternalOutput").ap()

    with tile.TileContext(nc) as tc:
        with (
            tc.tile_pool(name="const", bufs=1) as const,
            tc.tile_pool(name="apool", bufs=4) as apool,
            tc.tile_pool(name="ypool", bufs=8) as ypool,
            tc.tile_pool(name="mmp", bufs=3, space="PSUM") as mmp,
            tc.tile_pool(name="scp", bufs=2, space="PSUM") as scp,
            tc.tile_pool(name="otp", bufs=1, space="PSUM") as otp,
        ):
            # ---------------- DMAs (order = HWDGE priority) ----------------
            wk_sb = const.tile([128, NKT, 3 * CW], F16)
            xT_sb = const.tile([128, NKT, S], F16)
            misc_sb = const.tile([CW, 8 + 384 + HIDDEN], F16)

            head_sb = const.tile([128, 2 * 3 * CW + STRIP], F16)

            xr = xT  # dram layout already [128, NKT, S]
            # head bundle first (wk k0/k1 + xT k0-s0: the first 1.3us of PE
            # work in one DMA); miscA (biases+gkh) early; miscB later
            nc.sync.dma_start(head_sb, head)
            nc.scalar.dma_start(xT_sb[:, 1, ts(0, STRIP)], xr[:, 1, ts(0, STRIP)])
            nc.sync.dma_start(wk_sb[:, 2:4, :], wk[:, 2:4, :])
            nc.scalar.dma_start(misc_sb[:, 0:136], misc[:, 0:136])
            nc.sync.dma_start(xT_sb[:, 2, ts(0, STRIP)], xr[:, 2, ts(0, STRIP)])
            nc.scalar.dma_start(xT_sb[:, 3, ts(0, STRIP)], xr[:, 3, ts(0, STRIP)])
            nc.scalar.dma_start(misc_sb[:, 136:], misc[:, 136:])
            nc.sync.dma_start(xT_sb[:, :, ts(1, STRIP)], xr[:, :, ts(1, STRIP)])

            wkap = [head_sb[:, 0:384], head_sb[:, 384:768],
                    wk_sb[:, 2, :], wk_sb[:, 3, :]]
            x0ap = [head_sb[:, 768: 768 + STRIP], xT_sb[:, 1, ts(0, STRIP)],
                    xT_sb[:, 2, ts(0, STRIP)], xT_sb[:, 3, ts(0, STRIP)]]

            qkvb_sb = misc_sb[:, 0:6].bitcast(F32)      # [128, 3] f32
            qb_sb = qkvb_sb[:, 0:1]
            kb_sb = qkvb_sb[:, 1:2]
            vb_sb = qkvb_sb[:, 2:3]
            gkh_sb = misc_sb[:, 6:8].bitcast(F32)       # [128, 1] f32 g^(-p/2)
            gq_sb = misc_sb[:, 8:136]
            ident_sb = misc_sb[:, 136:264]
            dtri_sb = misc_sb[:, 264:392]
            ow_sb = misc_sb[:, 392: 392 + HIDDEN]



            qT_sb = const.tile([CW, S], F16, tag="qT")
            kT_sb = const.tile([CW, S], F16, tag="kT")
            kT2_sb = const.tile([CW, S], F16, tag="kT2")
            vT_sb = const.tile([CW, S], F16, tag="vT")
            v_sb = const.tile([128, NST, CW], F16, tag="v")
            outT_sb = const.tile([CW, S], F16, tag="outT")
            warm_sb = const.tile([128, WARM_ROWS], F16, tag="warm")

            # ------------- warmup: PE pstate ramp + Act table load ----------
            nc.gpsimd.memset(warm_sb, 0.0)
            warm_ps = mmp.tile([128, 512], F32, tag="mm", name="warm")
            for i in range(N_WARMUP):
                nc.tensor.matmul(warm_ps[:, 0:WARM_ROWS], warm_sb[:, 0:128],
                                 warm_sb[:, 0:WARM_ROWS], start=True, stop=True)
            # Act function-table load early (Square/Identity/Copy set)
            nc.scalar.square(warm_sb[:, 0:64], warm_sb[:, 64:128])



            # ---------------- QKV projection for one 512-strip --------------
            # per-ktile interleave: q,k,v consume each xT ktile as it lands
            def qkv_strip(T):
                tsl = ts(T, STRIP)
                g2 = lambda g: _free_bcast(g, STRIP // 128)

                # sequential proj groups: k closes first (scores path), then
                # q, then v — staggers the psum->SBUF copies across engines
                kps = mmp.tile([128, STRIP], F32, tag="mm", name=f"k{T}")
                qps = mmp.tile([128, STRIP], F32, tag="mm", name=f"q{T}")
                xv = lambda kk: x0ap[kk] if T == 0 else xT_sb[:, kk, tsl]
                # k/q interleaved per ktile: q matmuls fill the PE idle while
                # k waits for the next xT piece to land
                for kk in range(NKT):
                    nc.tensor.matmul(kps, wkap[kk][:, CW:2 * CW], xv(kk),
                                     start=kk == 0, stop=kk == NKT - 1)
                    nc.tensor.matmul(qps, wkap[kk][:, 0:CW], xv(kk),
                                     start=kk == 0, stop=kk == NKT - 1)
                # k decay g^(-si/2) is applied later as the Square's scale
                nc.scalar.activation(kT_sb[:, tsl], kps, AF.Identity, bias=kb_sb)
                nc.vector.scalar_tensor_tensor(
                    out=qT_sb[:, tsl].rearrange("p (a b) -> p a b", b=128),
                    in0=qps.rearrange("p (a b) -> p a b", b=128),
                    scalar=qb_sb, in1=g2(gq_sb), op0=OP.add, op1=OP.mult)
                # band stationary: kT2 = kT * g^64 (DVE 4x mode, all f16 SBUF)
                nc.vector.tensor_scalar_mul(kT2_sb[:, tsl], kT_sb[:, tsl], G64)
                vps = mmp.tile([128, STRIP], F32, tag="mm", name=f"v{T}")
                for kk in range(NKT):
                    nc.tensor.matmul(vps, wkap[kk][:, 2 * CW:3 * CW], xv(kk),
                                     start=kk == 0, stop=kk == NKT - 1)
                nc.scalar.activation(vT_sb[:, tsl], vps, AF.Identity,
                                     bias=vb_sb)

            def transpose_v(a):
                tp = mmp.tile([128, 128], F16, tag="mm", name=f"tp{a}")
                nc.tensor.transpose(tp, vT_sb[:, ts(a, 128)], ident_sb)
                nc.vector.tensor_copy(v_sb[:, a, :], tp)

            # ---------------- attention tile a ------------------------------
            oT = otp.tile([128, 2, 256], F32, tag="oT")  # one bank, 2 strips

            a4s = {}

            def att_sc(a, mask_dve=False):
                has_band = a < NST - 1
                n = WIN if has_band else 128
                w0 = 128 * a
                # scores psum: [128, 2, 512]-stride tile = 2 banks, head h
                # region [:, hh, :] gets its own bank (same-bank row-group
                # pairs hang)
                a4 = apool.tile([128, HPC, WIN], F16, tag="a4", name=f"a4_{a}")
                a4s[a] = a4
                for g, ps2 in ((0, scp.tile([128, 2, 512], F32, tag="sc",
                                            name=f"scA{a}")),
                               (1, scp.tile([128, 2, 512], F32, tag="sc",
                                            name=f"scB{a}"))):
                    for hh in range(2):
                        h = 2 * g + hh
                        hs = ts(h, 32)
                        # diag: stationary kT tile-a, moving qT same window
                        nc.tensor.matmul(
                            ps2[:, hh, 0:128], kT_sb[hs, ts(a, 128)],
                            qT_sb[hs, ts(a, 128)],
                            start=True, stop=True,
                            tile_position=(32 * h, 0))
                        if has_band:
                            # band: stationary kT2 (g^64 folded), moving qT of
                            # next tile's first 64 cols
                            nc.tensor.matmul(
                                ps2[:, hh, 128:WIN], kT2_sb[hs, ts(a, 128)],
                                qT_sb[hs, w0 + 128: w0 + WIN],
                                start=True, stop=True,
                                tile_position=(32 * h, 0),
                                skip_group_check=True)
                    nc.scalar.activation(a4[:, 2 * g: 2 * g + 2, 0:n],
                                         ps2[:, :, 0:n], AF.Square,
                                         scale=gkh_sb)
                    # zero the invalid (si > ti) upper part of the diag
                    # block; per head-group so av h01 starts after group 0
                    nc.gpsimd.affine_select(
                        a4[:, 2 * g: 2 * g + 2, 0:128],
                        a4[:, 2 * g: 2 * g + 2, 0:128],
                        pattern=[[0, 2], [1, 128]], compare_op=OP.is_ge,
                        fill=0.0, base=0, channel_multiplier=-1)

            # block j7 (t 896:1024) accumulates in a separate bank (borrowed
            # from the sc pool) so j6 and j7 close independently at the tail
            ot3b = {}

            def att_av(a):
                has_band = a < NST - 1
                a4 = a4s.pop(a)
                # A@V accumulation into oT strips (memset + pure accumulate)
                Td, cd = divmod(a, 2)     # diag -> strip Td cols [128*cd, +128)
                Tb, cb = divmod(a + 1, 2)  # band -> strip Tb cols [128*cb, +64)
                # band first: its a4 region needs no mask, so these matmuls
                # issue while the diag mask is still in flight
                if has_band:
                    bb = oT[:, Tb % 2, 128 * cb: 128 * cb + BAND]
                    if a == 6:
                        bb = ot3b["t"][:, 0:BAND]
                    for h in range(HPC):
                        hs = ts(h, 32)
                        nc.tensor.matmul(
                            bb[hs, :],
                            v_sb[:, a, hs], a4[:, h, 128:WIN], start=False,
                            stop=False,
                            tile_position=(0, 32 * h), skip_group_check=True)
                dd = oT[:, Td % 2, ts(cd, 128)]
                if a == 7:
                    dd = ot3b["t"][:, 0:128]
                for h in range(HPC):
                    hs = ts(h, 32)
                    nc.tensor.matmul(
                        dd[hs, :], v_sb[:, a, hs],
                        a4[:, h, 0:128], start=False,
                        stop=(h == HPC - 1 and a in (2 * Td + 1, 6, 7)),
                        tile_position=(0, 32 * h), skip_group_check=True)

            def memset_strip(T):
                nc.vector.memset(oT[:, T % 2, :], 0.0)

            def close_strip(T, split=False):
                base = 256 * T
                nc.vector.tensor_copy(outT_sb[:, base: base + 256],
                                      oT[:, T % 2, :])

            def outproj(j, qidx, copy="dve"):
                yps = mmp.tile([128, HIDDEN], F32, tag="mm", name=f"y{j}")
                nc.tensor.matmul(yps, outT_sb[:, ts(j, 128)], ow_sb,
                                 start=True, stop=True)
                y_sb = ypool.tile([128, HIDDEN], F16, tag="y")
                if copy == "split":
                    nc.vector.tensor_copy(y_sb[:, 0:256], yps[:, 0:256])
                    nc.scalar.copy(y_sb[:, 256:512], yps[:, 256:512])
                elif copy == "act":
                    nc.scalar.copy(y_sb, yps)
                else:
                    nc.vector.tensor_copy(y_sb, yps)
                q = nc.scalar if j == 6 else nc.sync
                q.dma_start(yp[ts(j, 128), :], y_sb)

            # ---------------- schedule ----------------
            # att split in two phases (sc -> av) with 2 tiles of lookahead so
            # the in-order PE queue hides the square->mask chain latency.
            memset_strip(0)
            memset_strip(1)
            qkv_strip(0)
            att_sc(0)
            att_sc(1)
            transpose_v(0)
            transpose_v(1)
            att_sc(2)
            transpose_v(2)
            transpose_v(3)
            att_av(0)
            qkv_strip(1)
            att_sc(3)
            att_av(1)
            close_strip(0)
            memset_strip(2)
            att_sc(4)
            outproj(0, 0)
            att_av(2)
            outproj(1, 1)
            transpose_v(4)
            transpose_v(5)
            transpose_v(6)
            transpose_v(7)
            att_sc(5)
            att_av(3)
            close_strip(1)
            memset_strip(3)
            att_sc(6)
            outproj(2, 0)
            att_av(4)
            outproj(3, 1)
            att_sc(7)
            att_av(5)
            close_strip(2)
            outproj(4, 0)
            outproj(5, 1)
            ot3b["t"] = mmp.tile([128, 512], F32, tag="mm", name="ot3b")
            nc.vector.memset(ot3b["t"][:, 0:128], 0.0)
            att_av(6)
            # close j6 (oT strip3 low half) independently of av(7)
            nc.scalar.copy(outT_sb[:, 768:896], oT[:, 1, 0:128])
            outproj(6, 1, copy="act")
            att_av(7)
            nc.vector.tensor_copy(outT_sb[:, 896:1024], ot3b["t"][:, 0:128])
            outproj(7, 0, copy="dve")

    nc.compile()
    return nc


_CACHED = None


def _get_program():
    global _CACHED
    if _CACHED is None:
        _CACHED = _build_program()
    return _CACHED


def _in_maps(x, q_w, q_b, k_w, k_b, v_w, v_b, o_w, o_b):
    x = np.asarray(x, np.float32)
    ti = np.arange(128, dtype=np.float64)
    gq = (GAMMA ** (ti / 2)).astype(np.float16)
    gq_rep = np.broadcast_to(gq, (CW, 128))

    qw_f, kw_f, vw_f = (np.asarray(w, np.float32) for w in (q_w, k_w, v_w))
    qb_f, kb_f, vb_f = (np.asarray(b, np.float32) for b in (q_b, k_b, v_b))
    ow_f = np.asarray(o_w, np.float32)
    ident = np.eye(128, dtype=np.float16)

    in_maps = []
    for c in range(NCORES):
        b, g = divmod(c, HPC)
        cs = slice(g * CW, (g + 1) * CW)
        # xT: [hidden, S] -> [NKT, 128, S] -> [128, NKT, S]
        xT_v = np.ascontiguousarray(
            x[b].T.reshape(NKT, 128, S).transpose(1, 0, 2).astype(np.float16))
        wk_v = np.concatenate([qw_f[:, cs], kw_f[:, cs], vw_f[:, cs]], axis=1)
        wk_v = np.ascontiguousarray(
            wk_v.reshape(NKT, 128, 3 * CW).astype(np.float16))
        wk_v = np.ascontiguousarray(wk_v.transpose(1, 0, 2))
        qkvb = np.stack([qb_f[cs], kb_f[cs], vb_f[cs]], axis=1).astype(np.float32)
        qkvb16 = qkvb.view(np.float16).reshape(CW, 6)
        gkh = (GAMMA ** (-np.arange(128, dtype=np.float64) / 2)).astype(
            np.float32)[:, None]
        gkh16 = gkh.view(np.float16).reshape(CW, 2)
        dtri = np.triu(np.ones((128, 128), np.float16))
        misc_v = np.ascontiguousarray(np.concatenate(
            [qkvb16, gkh16, gq_rep, ident, dtri,
             ow_f[cs, :].astype(np.float16)], axis=1))
        head_v = np.ascontiguousarray(np.concatenate(
            [wk_v[:, 0, :], wk_v[:, 1, :], xT_v[:, 0, 0:STRIP]], axis=1))
        in_maps.append({
            "xT": xT_v,
            "wk": wk_v,
            "misc": misc_v,
            "head": head_v,
        })
    return in_maps


def _gather(res, o_b):
    parts = [res.results[c]["yp"] for c in range(NCORES)]
    out = np.empty((B, S, HIDDEN), np.float32)
    ob = np.asarray(o_b, np.float32)
    for b in range(B):
        out[b] = (
            parts[4 * b].astype(np.float32)
            + parts[4 * b + 1].astype(np.float32)
            + parts[4 * b + 2].astype(np.float32)
            + parts[4 * b + 3].astype(np.float32)
            + ob
        )
    return out


def kernel(x, q_w, q_b, k_w, k_b, v_w, v_b, o_w, o_b):
    in_maps = _in_maps(x, q_w, q_b, k_w, k_b, v_w, v_b, o_w, o_b)
    nc = _get_program()
    res = bass_utils.run_bass_kernel_spmd(nc, in_maps, core_ids=list(range(NCORES)))
    return _gather(res, o_b)


def cost_model_time_ns():
    """Per-core makespan from the instruction cost model (no NTFF on axon)."""
    from concourse.timeline_sim import TimelineSim

    return TimelineSim(_get_program(), trace=False).simulate()


if __name__ == "__main__":
    rng = np.random.default_rng(0)
    lim = 1.0 / math.sqrt(HIDDEN)
    ins = {
        "x": rng.standard_normal((B, S, HIDDEN), dtype=np.float32),
        "q_w": rng.uniform(-lim, lim, (HIDDEN, HIDDEN)).astype(np.float32),
        "q_b": rng.uniform(-lim, lim, HIDDEN).astype(np.float32),
        "k_w": rng.uniform(-lim, lim, (HIDDEN, HIDDEN)).astype(np.float32),
        "k_b": rng.uniform(-lim, lim, HIDDEN).astype(np.float32),
        "v_w": rng.uniform(-lim, lim, (HIDDEN, HIDDEN)).astype(np.float32),
        "v_b": rng.uniform(-lim, lim, HIDDEN).astype(np.float32),
        "o_w": rng.uniform(-lim, lim, (HIDDEN, HIDDEN)).astype(np.float32),
        "o_b": rng.uniform(-lim, lim, HIDDEN).astype(np.float32),
    }
    out = kernel(**ins)
    print("kernel ran, out shape", out.shape, "norm", np.linalg.norm(out))


# revision 93
# speedup vs baseline: 1.0018x; 1.0002x over previous
"""Power attention (p=2) layer for Trainium2, 8 NeuronCores.

Math: spow2(q).spow2(k) = (q.k)^2, so the reference recurrence equals
    out_t = sum_{s<=t} g^(t-s) (q_t.k_s)^2 v_s   (masked quadratic attention).

Decay: LOCAL per-128-tile scaling. With ti = t mod 128, si = s mod 128:
  qT cols scaled by g^(ti/2), kT cols by g^(-si/2)  ->  diag tile products
  carry exactly g^(ti-si) after squaring. The band tile (k-tile a vs query
  tile a+1, first 64 query cols) uses kT2 = kT * g^64, giving g^(128+tb-si)
  after squaring. Window 192 per k-tile: dropped terms decay <= g^65 ~ 1e-3.

Upper-tri invalid pairs of the diag tile are squared (may overflow f16 to
inf harmlessly) then REPLACED with 0 by a gpsimd affine_select.

Sharding: core c -> batch c//4, head group c%4 (4 heads = 128 qkv cols).
Host sums the 4 partial [S, HIDDEN] outputs per batch and adds o_b.

All DRAM traffic is f16; PSUM accumulation f32.
"""

import math
import sys

import numpy as np

sys.path.insert(0, "/opt/trn_rl_repo")

import concourse.bass as bass  # noqa: E402
import concourse.tile as tile  # noqa: E402
from concourse import bacc  # noqa: E402
from concourse import mybir  # noqa: E402
from concourse import bass_utils  # noqa: E402
from concourse.bass import ts  # noqa: E402

B, S, HIDDEN = 2, 1024, 512
NH, HD = 16, 32
GAMMA = 0.9
NCORES = 8
HPC = 4            # heads per core
CW = HPC * HD      # 128 qkv columns per core
NKT = HIDDEN // 128  # 4 contraction tiles
NST = S // 128       # 8 seq tiles
STRIP = 512
NSTRIP = S // STRIP  # 2
BAND = 64            # band width beyond the diag tile (window = 128+BAND)
WIN = 128 + BAND

G64 = float(GAMMA) ** 64

F32 = mybir.dt.float32
F16 = mybir.dt.float16
AF = mybir.ActivationFunctionType
OP = mybir.AluOpType

# ---- tuning knobs ----
N_WARMUP = 1          # warmup matmuls of 512 rows each (pstate ramp cover)
WARM_ROWS = 128


def _free_bcast(ap2d, times):
    """[P, N] AP -> [P, times, N] AP broadcasting along a new middle free dim."""
    part, free = ap2d.ap[0], list(ap2d.ap[1:])
    return bass.AP(tensor=ap2d.tensor, offset=ap2d.offset,
                   ap=[part, [0, times]] + free)


def _bcast_dram(ap, p=128):
    """DRAM-side AP replicating a [1, ...] tensor across p partitions."""
    return bass.AP(tensor=ap.tensor, offset=ap.offset,
                   ap=[[0, p]] + list(ap.ap[1:]))


def _build_program():
    nc = bacc.Bacc("TRN2", debug=False, target_bir_lowering=False)

    # DRAM inputs (per core, host-prepared, all f16)
    xT = nc.dram_tensor("xT", [128, NKT, S], F16, kind="ExternalInput").ap()
    wk = nc.dram_tensor("wk", [128, NKT, 3 * CW], F16, kind="ExternalInput").ap()
    # misc: [qkvb(6: f32 qb,kb,vb bitcast) | gkh(2: f32 g^(-si/2) bitcast)
    #        | ident(128) | dtri(128) | ow(512)]
    misc = nc.dram_tensor("misc", [CW, 8 + 384 + HIDDEN], F16,
                          kind="ExternalInput").ap()
    # head: first-use bundle {wk ktile0 | wk ktile1 | xT ktile0 strip0}
    head = nc.dram_tensor("head", [128, 2 * 3 * CW + STRIP], F16,
                          kind="ExternalInput").ap()
    yp = nc.dram_tensor("yp", [S, HIDDEN], F16, kind="ExternalOutput").ap()

    with tile.TileContext(nc) as tc:
        with (
            tc.tile_pool(name="const", bufs=1) as const,
            tc.tile_pool(name="apool", bufs=4) as apool,
            tc.tile_pool(name="ypool", bufs=8) as ypool,
            tc.tile_pool(name="mmp", bufs=3, space="PSUM") as mmp,
            tc.tile_pool(name="scp", bufs=2, space="PSUM") as scp,
            tc.tile_pool(name="otp", bufs=1, space="PSUM") as otp,
        ):
            # ---------------- DMAs (order = HWDGE priority) ----------------
            wk_sb = const.tile([128, NKT, 3 * CW], F16)
            xT_sb = const.tile([128, NKT, S], F16)
            misc_sb = const.tile([CW, 8 + 384 + HIDDEN], F16)

            head_sb = const.tile([128, 2 * 3 * CW + STRIP], F16)

            xr = xT  # dram layout already [128, NKT, S]
            # head bundle first (wk k0/k1 + xT k0-s0: the first 1.3us of PE
            # work in one DMA); miscA (biases+gkh) early; miscB later
            nc.sync.dma_start(head_sb, head)
            nc.scalar.dma_start(xT_sb[:, 1, ts(0, STRIP)], xr[:, 1, ts(0, STRIP)])
            nc.sync.dma_start(wk_sb[:, 2:4, :], wk[:, 2:4, :])
            nc.scalar.dma_start(misc_sb[:, 0:136], misc[:, 0:136])
            nc.sync.dma_start(xT_sb[:, 2, ts(0, STRIP)], xr[:, 2, ts(0, STRIP)])
            nc.scalar.dma_start(xT_sb[:, 3, ts(0, STRIP)], xr[:, 3, ts(0, STRIP)])
            nc.scalar.dma_start(misc_sb[:, 136:], misc[:, 136:])
            nc.sync.dma_start(xT_sb[:, :, ts(1, STRIP)], xr[:, :, ts(1, STRIP)])

            wkap = [head_sb[:, 0:384], head_sb[:, 384:768],
                    wk_sb[:, 2, :], wk_sb[:, 3, :]]
            x0ap = [head_sb[:, 768: 768 + STRIP], xT_sb[:, 1, ts(0, STRIP)],
                    xT_sb[:, 2, ts(0, STRIP)], xT_sb[:, 3, ts(0, STRIP)]]

            qkvb_sb = misc_sb[:, 0:6].bitcast(F32)      # [128, 3] f32
            qb_sb = qkvb_sb[:, 0:1]
            kb_sb = qkvb_sb[:, 1:2]
            vb_sb = qkvb_sb[:, 2:3]
            gkh_sb = misc_sb[:, 6:8].bitcast(F32)       # [128, 1] f32 g^(-p/2)
            gq_sb = misc_sb[:, 8:136]
            ident_sb = misc_sb[:, 136:264]
            dtri_sb = misc_sb[:, 264:392]
            ow_sb = misc_sb[:, 392: 392 + HIDDEN]



            qT_sb = const.tile([CW, S], F16, tag="qT")
            kT_sb = const.tile([CW, S], F16, tag="kT")
            kT2_sb = const.tile([CW, S], F16, tag="kT2")
            vT_sb = const.tile([CW, S], F16, tag="vT")
            v_sb = const.tile([128, NST, CW], F16, tag="v")
            outT_sb = const.tile([CW, S], F16, tag="outT")
            warm_sb = const.tile([128, WARM_ROWS], F16, tag="warm")

            # ------------- warmup: PE pstate ramp + Act table load ----------
            nc.gpsimd.memset(warm_sb, 0.0)
            warm_ps = mmp.tile([128, 512], F32, tag="mm", name="warm")
            for i in range(N_WARMUP):
                nc.tensor.matmul(warm_ps[:, 0:WARM_ROWS], warm_sb[:, 0:128],
                                 warm_sb[:, 0:WARM_ROWS], start=True, stop=True)
            # Act function-table load early (Square/Identity/Copy set)
            nc.scalar.square(warm_sb[:, 0:64], warm_sb[:, 64:128])



            # ---------------- QKV projection for one 512-strip --------------
            # per-ktile interleave: q,k,v consume each xT ktile as it lands
            def qkv_strip(T):
                tsl = ts(T, STRIP)
                g2 = lambda g: _free_bcast(g, STRIP // 128)

                # sequential proj groups: k closes first (scores path), then
                # q, then v — staggers the psum->SBUF copies across engines
                kps = mmp.tile([128, STRIP], F32, tag="mm", name=f"k{T}")
                qps = mmp.tile([128, STRIP], F32, tag="mm", name=f"q{T}")
                xv = lambda kk: x0ap[kk] if T == 0 else xT_sb[:, kk, tsl]
                vps = mmp.tile([128, STRIP], F32, tag="mm", name=f"v{T}")
                # k/q/v interleaved per ktile: later-group matmuls fill the PE
                # idle while k waits for the next xT piece to land
                for kk in range(NKT):
                    nc.tensor.matmul(kps, wkap[kk][:, CW:2 * CW], xv(kk),
                                     start=kk == 0, stop=kk == NKT - 1)
                    nc.tensor.matmul(qps, wkap[kk][:, 0:CW], xv(kk),
                                     start=kk == 0, stop=kk == NKT - 1)
                    nc.tensor.matmul(vps, wkap[kk][:, 2 * CW:3 * CW], xv(kk),
                                     start=kk == 0, stop=kk == NKT - 1)
                # k decay g^(-si/2) is applied later as the Square's scale
                nc.scalar.activation(kT_sb[:, tsl], kps, AF.Identity, bias=kb_sb)
                nc.vector.scalar_tensor_tensor(
                    out=qT_sb[:, tsl].rearrange("p (a b) -> p a b", b=128),
                    in0=qps.rearrange("p (a b) -> p a b", b=128),
                    scalar=qb_sb, in1=g2(gq_sb), op0=OP.add, op1=OP.mult)
                # band stationary: kT2 = kT * g^64 (DVE 4x mode, all f16 SBUF)
                nc.vector.tensor_scalar_mul(kT2_sb[:, tsl], kT_sb[:, tsl], G64)
                nc.scalar.activation(vT_sb[:, tsl], vps, AF.Identity,
                                     bias=vb_sb)

            def transpose_v(a):
                tp = mmp.tile([128, 128], F16, tag="mm", name=f"tp{a}")
                nc.tensor.transpose(tp, vT_sb[:, ts(a, 128)], ident_sb)
                nc.vector.tensor_copy(v_sb[:, a, :], tp)

            # ---------------- attention tile a ------------------------------
            oT = otp.tile([128, 2, 256], F32, tag="oT")  # one bank, 2 strips

            a4s = {}

            def att_sc(a, mask_dve=False):
                has_band = a < NST - 1
                n = WIN if has_band else 128
                w0 = 128 * a
                # scores psum: [128, 2, 512]-stride tile = 2 banks, head h
                # region [:, hh, :] gets its own bank (same-bank row-group
                # pairs hang)
                a4 = apool.tile([128, HPC, WIN], F16, tag="a4", name=f"a4_{a}")
                a4s[a] = a4
                for g, ps2 in ((0, scp.tile([128, 2, 512], F32, tag="sc",
                                            name=f"scA{a}")),
                               (1, scp.tile([128, 2, 512], F32, tag="sc",
                                            name=f"scB{a}"))):
                    for hh in range(2):
                        h = 2 * g + hh
                        hs = ts(h, 32)
                        # diag: stationary kT tile-a, moving qT same window
                        nc.tensor.matmul(
                            ps2[:, hh, 0:128], kT_sb[hs, ts(a, 128)],
                            qT_sb[hs, ts(a, 128)],
                            start=True, stop=True,
                            tile_position=(32 * h, 0))
                        if has_band:
                            # band: stationary kT2 (g^64 folded), moving qT of
                            # next tile's first 64 cols
                            nc.tensor.matmul(
                                ps2[:, hh, 128:WIN], kT2_sb[hs, ts(a, 128)],
                                qT_sb[hs, w0 + 128: w0 + WIN],
                                start=True, stop=True,
                                tile_position=(32 * h, 0),
                                skip_group_check=True)
                    nc.scalar.activation(a4[:, 2 * g: 2 * g + 2, 0:n],
                                         ps2[:, :, 0:n], AF.Square,
                                         scale=gkh_sb)
                    # zero the invalid (si > ti) upper part of the diag
                    # block; per head-group so av h01 starts after group 0
                    nc.gpsimd.affine_select(
                        a4[:, 2 * g: 2 * g + 2, 0:128],
                        a4[:, 2 * g: 2 * g + 2, 0:128],
                        pattern=[[0, 2], [1, 128]], compare_op=OP.is_ge,
                        fill=0.0, base=0, channel_multiplier=-1)

            # block j7 (t 896:1024) accumulates in a separate bank (borrowed
            # from the sc pool) so j6 and j7 close independently at the tail
            ot3b = {}

            def att_av(a):
                has_band = a < NST - 1
                a4 = a4s.pop(a)
                # A@V accumulation into oT strips (memset + pure accumulate)
                Td, cd = divmod(a, 2)     # diag -> strip Td cols [128*cd, +128)
                Tb, cb = divmod(a + 1, 2)  # band -> strip Tb cols [128*cb, +64)
                # band first: its a4 region needs no mask, so these matmuls
                # issue while the diag mask is still in flight
                if has_band:
                    bb = oT[:, Tb % 2, 128 * cb: 128 * cb + BAND]
                    if a == 6:
                        bb = ot3b["t"][:, 0:BAND]
                    for h in range(HPC):
                        hs = ts(h, 32)
                        nc.tensor.matmul(
                            bb[hs, :],
                            v_sb[:, a, hs], a4[:, h, 128:WIN], start=False,
                            stop=False,
                            tile_position=(0, 32 * h), skip_group_check=True)
                dd = oT[:, Td % 2, ts(cd, 128)]
                if a == 7:
                    dd = ot3b["t"][:, 0:128]
                for h in range(HPC):
                    hs = ts(h, 32)
                    nc.tensor.matmul(
                        dd[hs, :], v_sb[:, a, hs],
                        a4[:, h, 0:128], start=False,
                        stop=(h == HPC - 1 and a in (2 * Td + 1, 6, 7)),
                        tile_position=(0, 32 * h), skip_group_check=True)

            def memset_strip(T):
                nc.vector.memset(oT[:, T % 2, :], 0.0)

            def close_strip(T, split=False):
                base = 256 * T
                nc.vector.tensor_copy(outT_sb[:, base: base + 256],
                                      oT[:, T % 2, :])

            def outproj(j, qidx, copy="dve"):
                yps = mmp.tile([128, HIDDEN], F32, tag="mm", name=f"y{j}")
                nc.tensor.matmul(yps, outT_sb[:, ts(j, 128)], ow_sb,
                                 start=True, stop=True)
                y_sb = ypool.tile([128, HIDDEN], F16, tag="y")
                if copy == "split":
                    nc.vector.tensor_copy(y_sb[:, 0:256], yps[:, 0:256])
                    nc.scalar.copy(y_sb[:, 256:512], yps[:, 256:512])
                elif copy == "act":
                    nc.scalar.copy(y_sb, yps)
                else:
                    nc.vector.tensor_copy(y_sb, yps)
                q = nc.scalar if j == 6 else nc.sync
                q.dma_start(yp[ts(j, 128), :], y_sb)

            # ---------------- schedule ----------------
            # att split in two phases (sc -> av) with 2 tiles of lookahead so
            # the in-order PE queue hides the square->mask chain latency.
            memset_strip(0)
            memset_strip(1)
            qkv_strip(0)
            att_sc(0)
            att_sc(1)
            transpose_v(0)
            transpose_v(1)
            att_sc(2)
            transpose_v(2)
            transpose_v(3)
            att_av(0)
            qkv_strip(1)
            att_sc(3)
            att_av(1)
            close_strip(0)
            memset_strip(2)
            att_sc(4)
            outproj(0, 0)
            att_av(2)
            outproj(1, 1)
            transpose_v(4)
            transpose_v(5)
            transpose_v(6)
            transpose_v(7)
            att_sc(5)
            att_av(3)
            close_strip(1)
            memset_strip(3)
            att_sc(6)
            outproj(2, 0)
            att_av(4)
            outproj(3, 1)
            att_sc(7)
            att_av(5)
            close_strip(2)
            outproj(4, 0)
            outproj(5, 1)
            ot3b["t"] = mmp.tile([128, 512], F32, tag="mm", name="ot3b")
            nc.vector.memset(ot3b["t"][:, 0:128], 0.0)
            att_av(6)
            # close j6 (oT strip3 low half) independently of av(7)
            nc.scalar.copy(outT_sb[:, 768:896], oT[:, 1, 0:128])
            outproj(6, 1, copy="act")
            att_av(7)
            nc.vector.tensor_copy(outT_sb[:, 896:1024], ot3b["t"][:, 0:128])
            outproj(7, 0, copy="dve")

    nc.compile()
    return nc


_CACHED = None


def _get_program():
    global _CACHED
    if _CACHED is None:
        _CACHED = _build_program()
    return _CACHED


def _in_maps(x, q_w, q_b, k_w, k_b, v_w, v_b, o_w, o_b):
    x = np.asarray(x, np.float32)
    ti = np.arange(128, dtype=np.float64)
    gq = (GAMMA ** (ti / 2)).astype(np.float16)
    gq_rep = np.broadcast_to(gq, (CW, 128))

    qw_f, kw_f, vw_f = (np.asarray(w, np.float32) for w in (q_w, k_w, v_w))
    qb_f, kb_f, vb_f = (np.asarray(b, np.float32) for b in (q_b, k_b, v_b))
    ow_f = np.asarray(o_w, np.float32)
    ident = np.eye(128, dtype=np.float16)

    in_maps = []
    for c in range(NCORES):
        b, g = divmod(c, HPC)
        cs = slice(g * CW, (g + 1) * CW)
        # xT: [hidden, S] -> [NKT, 128, S] -> [128, NKT, S]
        xT_v = np.ascontiguousarray(
            x[b].T.reshape(NKT, 128, S).transpose(1, 0, 2).astype(np.float16))
        wk_v = np.concatenate([qw_f[:, cs], kw_f[:, cs], vw_f[:, cs]], axis=1)
        wk_v = np.ascontiguousarray(
            wk_v.reshape(NKT, 128, 3 * CW).astype(np.float16))
        wk_v = np.ascontiguousarray(wk_v.transpose(1, 0, 2))
        qkvb = np.stack([qb_f[cs], kb_f[cs], vb_f[cs]], axis=1).astype(np.float32)
        qkvb16 = qkvb.view(np.float16).reshape(CW, 6)
        gkh = (GAMMA ** (-np.arange(128, dtype=np.float64) / 2)).astype(
            np.float32)[:, None]
        gkh16 = gkh.view(np.float16).reshape(CW, 2)
        dtri = np.triu(np.ones((128, 128), np.float16))
        misc_v = np.ascontiguousarray(np.concatenate(
            [qkvb16, gkh16, gq_rep, ident, dtri,
             ow_f[cs, :].astype(np.float16)], axis=1))
        head_v = np.ascontiguousarray(np.concatenate(
            [wk_v[:, 0, :], wk_v[:, 1, :], xT_v[:, 0, 0:STRIP]], axis=1))
        in_maps.append({
            "xT": xT_v,
            "wk": wk_v,
            "misc": misc_v,
            "head": head_v,
        })
    return in_maps


def _gather(res, o_b):
    parts = [res.results[c]["yp"] for c in range(NCORES)]
    out = np.empty((B, S, HIDDEN), np.float32)
    ob = np.asarray(o_b, np.float32)
    for b in range(B):
        out[b] = (
            parts[4 * b].astype(np.float32)
            + parts[4 * b + 1].astype(np.float32)
            + parts[4 * b + 2].astype(np.float32)
            + parts[4 * b + 3].astype(np.float32)
            + ob
        )
    return out


def kernel(x, q_w, q_b, k_w, k_b, v_w, v_b, o_w, o_b):
    in_maps = _in_maps(x, q_w, q_b, k_w, k_b, v_w, v_b, o_w, o_b)
    nc = _get_program()
    res = bass_utils.run_bass_kernel_spmd(nc, in_maps, core_ids=list(range(NCORES)))
    return _gather(res, o_b)


def cost_model_time_ns():
    """Per-core makespan from the instruction cost model (no NTFF on axon)."""
    from concourse.timeline_sim import TimelineSim

    return TimelineSim(_get_program(), trace=False).simulate()


if __name__ == "__main__":
    rng = np.random.default_rng(0)
    lim = 1.0 / math.sqrt(HIDDEN)
    ins = {
        "x": rng.standard_normal((B, S, HIDDEN), dtype=np.float32),
        "q_w": rng.uniform(-lim, lim, (HIDDEN, HIDDEN)).astype(np.float32),
        "q_b": rng.uniform(-lim, lim, HIDDEN).astype(np.float32),
        "k_w": rng.uniform(-lim, lim, (HIDDEN, HIDDEN)).astype(np.float32),
        "k_b": rng.uniform(-lim, lim, HIDDEN).astype(np.float32),
        "v_w": rng.uniform(-lim, lim, (HIDDEN, HIDDEN)).astype(np.float32),
        "v_b": rng.uniform(-lim, lim, HIDDEN).astype(np.float32),
        "o_w": rng.uniform(-lim, lim, (HIDDEN, HIDDEN)).astype(np.float32),
        "o_b": rng.uniform(-lim, lim, HIDDEN).astype(np.float32),
    }
    out = kernel(**ins)
    print("kernel ran, out shape", out.shape, "norm", np.linalg.norm(out))
